# revision 1
# baseline (speedup 1.0000x reference)
"""Trainium2 Bass kernel for nn_CorrectorEGNN (B=128 graphs, N=64 nodes, H=128, L=4).

Strategy: data-parallel over graphs (16 graphs/core x 8 cores). Per graph the
fully-connected edge set is treated densely as 64x64 ordered pairs (i-major,
e = i*64+j, src=i, dst=j; the i==j diagonal is masked after the edge MLP).
All edge tensors live channel-transposed in SBUF: [128 chan (partitions),
4096 edges (free)] so the edge MLP is weight-stationary matmuls.

Edge-MLP first layer is restructured:
  e_in @ W1 = A'[src] + B'[dst] - 2*wrow x Gram
with A' = H@W1a + |p|^2*wrow, B' = H@W1b + |p|^2*wrow, so it becomes one
K=128 matmul against a constant 0/1 "selection" matrix S plus one K=1
rank-1 matmul against the flattened Gram row. Segment sums:
  msg_j = sum_i m[:, (i,j)]  -> DVE strided tensor_reduce
  pos update -> cw matrix [64,64] (via DRAM bounce) @ [P|1] one tiny matmul.
"""

import sys

sys.path.insert(0, "/opt/trn_rl_repo")

import numpy as np

N = 64
C = 3
H = 128
L = 4
B = 128
NCORES = 8
GPC = B // NCORES  # graphs per core
E = N * N  # dense edges per graph

_CACHE = {}


def _prep_consts(inputs):
    """Numpy-side packing of weights into DMA-friendly layouts (replicated per core)."""
    f32 = np.float32
    ew1 = np.asarray(inputs["edge_w1"], f32)  # [L, 2H+1, H]
    d = {}
    d["w1a"] = np.concatenate([ew1[l, :H] for l in range(L)], axis=1)  # [128, 512]
    d["w1b"] = np.concatenate([ew1[l, H : 2 * H] for l in range(L)], axis=1)
    wrow = ew1[:, 2 * H]  # [L, 128]
    d["wrep"] = np.concatenate(
        [np.tile(wrow[l][None, :], (N, 1)) for l in range(L)], axis=1
    )  # [64, 512]
    d["wm2"] = np.concatenate([(-2.0 * wrow[l])[None, :] for l in range(L)], axis=1)  # [1, 512]
    d["w2"] = np.concatenate([np.asarray(inputs["edge_w2"], f32)[l] for l in range(L)], axis=1)
    d["cw1"] = np.concatenate([np.asarray(inputs["coord_w1"], f32)[l] for l in range(L)], axis=1)
    d["cw2"] = np.concatenate([np.asarray(inputs["coord_w2"], f32)[l] for l in range(L)], axis=1)  # [128, 4]
    nw1 = np.asarray(inputs["node_w1"], f32)
    d["nw1a"] = np.concatenate([nw1[l, :H] for l in range(L)], axis=1)
    d["nw1b"] = np.concatenate([nw1[l, H:] for l in range(L)], axis=1)
    d["nw2"] = np.concatenate([np.asarray(inputs["node_w2"], f32)[l] for l in range(L)], axis=1)
    # biases: [128, 5*L]; column order b1(l), b2(l), cb1(l), nb1(l), nb2(l) interleaved per name
    bias_cols = []
    for nm in ("edge_b1", "edge_b2", "coord_b1", "node_b1", "node_b2"):
        arr = np.asarray(inputs[nm], f32)  # [L, 128]
        for l in range(L):
            bias_cols.append(arr[l][:, None])
    d["biases"] = np.concatenate(bias_cols, axis=1)  # [128, 20]
    d["nerep"] = np.tile(np.asarray(inputs["node_embed"], f32).T, (1, N))  # [128, 64]
    d["ident"] = np.eye(N, dtype=f32)
    os_val = float(np.asarray(inputs["output_scale"], f32)[0])
    msc = np.zeros((N, 2), f32)
    msc[:, 0] = 1.0
    msc[:, 1] = os_val
    d["msc"] = msc
    d["inv64"] = np.full((1, N), 1.0 / N, f32)
    # selection matrix S [128, E]: rows 0-63 pick src i, rows 64-127 pick dst j
    S = np.zeros((2 * N, E), f32)
    ii = np.repeat(np.arange(N), N)
    jj = np.tile(np.arange(N), N)
    S[ii, np.arange(E)] = 1.0
    S[N + jj, np.arange(E)] = 1.0
    d["S"] = S
    return d


def _build(n_graphs, num_devices, sim_silu=False):
    import concourse.bacc as bacc
    import concourse.tile as tile
    import concourse.mybir as mybir
    from concourse.bass import AP  # noqa: F401

    dt = mybir.dt
    f32 = dt.float32
    f32r = dt.float32r
    Silu = mybir.ActivationFunctionType.Silu
    add = mybir.AluOpType.add
    sub = mybir.AluOpType.subtract
    mult = mybir.AluOpType.mult
    AX = mybir.AxisListType.X

    nc = bacc.Bacc("TRN2", num_devices=num_devices, enable_partition_id=False)

    dr = {}
    for name, shape in [
        ("xin", [n_graphs, N, C]),
        ("xtin", [n_graphs, C, N]),
        ("S", [2 * N, E]),
        ("w1a", [H, L * H]),
        ("w1b", [H, L * H]),
        ("wrep", [N, L * H]),
        ("wm2", [1, L * H]),
        ("w2", [H, L * H]),
        ("cw1", [H, L * H]),
        ("cw2", [H, L]),
        ("nw1a", [H, L * H]),
        ("nw1b", [H, L * H]),
        ("nw2", [H, L * H]),
        ("biases", [H, 5 * L]),
        ("nerep", [H, N]),
        ("ident", [N, N]),
        ("msc", [N, 2]),
        ("inv64", [1, N]),
    ]:
        dr[name] = nc.dram_tensor(name, shape, f32, kind="ExternalInput").ap()
    y = nc.dram_tensor("y", [n_graphs, N, C], f32, kind="ExternalOutput").ap()

    def r(ap):
        return ap.bitcast(f32r)

    Sigmoid = mybir.ActivationFunctionType.Sigmoid

    from contextlib import ExitStack
    with nc.allow_low_precision(reason="fp32r matmul inputs"), tile.TileContext(nc) as tc, ExitStack() as es:
        cp = es.enter_context(tc.tile_pool(name="const", bufs=1))
        sp = es.enter_context(tc.tile_pool(name="state", bufs=1))
        wp = es.enter_context(tc.tile_pool(name="work", bufs=2))
        pp = es.enter_context(tc.tile_pool(name="psum", bufs=2, space="PSUM"))
        sm = es.enter_context(tc.tile_pool(name="smps", bufs=4, space="PSUM"))
        dp = es.enter_context(tc.tile_pool(name="dram", bufs=3, space="DRAM"))

        # ---- load constants ----
        ct = {}
        F32R_CONSTS = {"S", "w1a", "w1b", "wm2", "w2", "cw1", "cw2", "nw1a", "nw1b", "nw2"}
        for name in (
            "S", "w1a", "w1b", "wrep", "wm2", "w2", "cw1", "cw2",
            "nw1a", "nw1b", "nw2", "biases", "nerep", "ident", "msc", "inv64",
        ):
            cdt = f32r if name in F32R_CONSTS else f32
            t = cp.tile(list(dr[name].shape), cdt, tag=f"c_{name}")
            nc.sync.dma_start(out=t[:], in_=dr[name].bitcast(cdt) if cdt is f32r else dr[name])
            ct[name] = t

        zrow = cp.tile([1, N], f32, tag="zrow")
        nc.vector.memset(zrow[:], 0.0)
        ct["zrow"] = zrow

        def act_silu(out_ap, in_ap, bias_ap, tagz):
            if not sim_silu:
                nc.scalar.activation(out=out_ap, in_=in_ap, func=Silu, bias=bias_ap)
            else:
                z = wp.tile([out_ap.shape[0], out_ap.shape[1]], f32, tag=tagz)
                nc.vector.tensor_scalar_add(out=z[:], in0=in_ap, scalar1=bias_ap)
                nc.scalar.activation(out=out_ap, in_=z[:], func=Sigmoid)
                nc.vector.tensor_tensor(out=out_ap, in0=out_ap, in1=z[:], op=mult)

        def wsl(name, l):  # [128,128] weight slice of layer l
            return ct[name][:, l * H : (l + 1) * H]

        def bsl(bi, l):  # bias column [128,1]
            return ct["biases"][:, bi * L + l : bi * L + l + 1]

        # ---- per-graph state ----
        HTs, Pxs, PTs = [], [], []
        for g in range(n_graphs):
            HT = sp.tile([H, N], f32r, tag=f"HT{g}")
            nc.sync.dma_start(out=HT[:], in_=ct["nerep"][:].bitcast(f32r))
            Px = sp.tile([N, 4], f32, tag=f"Px{g}")
            nc.sync.dma_start(out=Px[:, 0:3], in_=dr["xin"][g])
            nc.vector.memset(Px[:, 3:4], 1.0)
            PT = sp.tile([C, N], f32, tag=f"PT{g}")
            nc.sync.dma_start(out=PT[:], in_=dr["xtin"][g])
            HTs.append(HT)
            Pxs.append(Px)
            PTs.append(PT)

        for g in range(n_graphs):
            HT, Px, PT = HTs[g], Pxs[g], PTs[g]
            for l in range(L):
                # A^T/B^T = H @ W1a|W1b  -> one [128,128] psum (B^T at partitions 64+)
                ab1 = sm.tile([N, H], f32, tag="sm")
                nc.tensor.matmul(out=ab1[:], lhsT=r(HT[:]), rhs=r(wsl("w1a", l)), start=True, stop=True)
                ab2 = sm.tile([N, H], f32, tag="sm")
                nc.tensor.matmul(out=ab2[:], lhsT=r(HT[:]), rhs=r(wsl("w1b", l)), start=True, stop=True)
                # gd = |p|^2 per node
                sq = wp.tile([N, C], f32, tag="sq")
                nc.vector.tensor_tensor(out=sq[:], in0=Px[:, 0:3], in1=Px[:, 0:3], op=mult)
                gd = wp.tile([N, 1], f32, tag="gd")
                nc.vector.tensor_reduce(out=gd[:], in_=sq[:], axis=AX, op=add)
                # lhsT_S = [A^T;B^T] + gd*wrow
                lS = wp.tile([2 * N, H], f32r, tag="lS")
                nc.vector.tensor_scalar_mul(out=lS[0:N, :], in0=ct["wrep"][:, l * H : (l + 1) * H], scalar1=gd[:])
                nc.vector.tensor_copy(out=lS[N:, :], in_=lS[0:N, :])
                nc.vector.tensor_tensor(out=lS[0:N, :], in0=lS[0:N, :], in1=ab1[:], op=add)
                nc.vector.tensor_tensor(out=lS[N:, :], in0=lS[N:, :], in1=ab2[:], op=add)
                # Gram -> DRAM bounce -> g_row [1, 4096]
                gram = sm.tile([N, N], f32, tag="sm")
                nc.tensor.matmul(out=gram[:], lhsT=PT[:], rhs=PT[:], start=True, stop=True)
                gramS = wp.tile([N, N], f32r, tag="gramS")
                nc.vector.tensor_copy(out=gramS[:], in_=gram[:])
                dgram = dp.tile([N, N], f32r, tag="dgram")
                nc.sync.dma_start(out=dgram[:], in_=gramS[:])
                g_row = wp.tile([1, E], f32r, tag="g_row")
                nc.sync.dma_start(out=g_row[:], in_=dgram[:].rearrange("a b -> (a b)")[None, :])

                # edge MLP over 4 tiles of 1024 edges
                t1 = wp.tile([H, E], f32r, tag="t1")
                for t in range(4):
                    ps = pp.tile([H, 1024], f32, tag="big")
                    for q in range(2):
                        c0 = t * 1024 + q * 512
                        nc.tensor.matmul(out=ps[:, q * 512 : (q + 1) * 512], lhsT=r(lS[:]),
                                         rhs=r(ct["S"][:, c0 : c0 + 512]), start=True, stop=False)
                        nc.tensor.matmul(out=ps[:, q * 512 : (q + 1) * 512], lhsT=r(ct["wm2"][:, l * H : (l + 1) * H]),
                                         rhs=r(g_row[:, c0 : c0 + 512]), start=False, stop=True)
                    act_silu(t1[:, t * 1024 : (t + 1) * 1024], ps[:], bsl(0, l), "z1")
                m = wp.tile([H, E], f32r, tag="m")
                for t in range(4):
                    ps = pp.tile([H, 1024], f32, tag="big")
                    for q in range(2):
                        c0 = t * 1024 + q * 512
                        nc.tensor.matmul(out=ps[:, q * 512 : (q + 1) * 512], lhsT=r(wsl("w2", l)),
                                         rhs=r(t1[:, c0 : c0 + 512]), start=True, stop=True)
                    act_silu(m[:, t * 1024 : (t + 1) * 1024], ps[:], bsl(1, l), "z1")
                t2 = wp.tile([H, E], f32r, tag="t2")
                for t in range(4):
                    ps = pp.tile([H, 1024], f32, tag="big")
                    for q in range(2):
                        c0 = t * 1024 + q * 512
                        nc.tensor.matmul(out=ps[:, q * 512 : (q + 1) * 512], lhsT=r(wsl("cw1", l)),
                                         rhs=r(m[:, c0 : c0 + 512]), start=True, stop=True)
                    act_silu(t2[:, t * 1024 : (t + 1) * 1024], ps[:], bsl(2, l), "z1")

                # cw row: [1, 4096] then bounce to CWM [64(i), 64(j)]
                cwr = wp.tile([1, E], f32, tag="cwr")
                for c in range(8):
                    cps = sm.tile([1, 512], f32, tag="sm")
                    nc.tensor.matmul(out=cps[:], lhsT=r(ct["cw2"][:, l : l + 1]),
                                     rhs=r(t2[:, c * 512 : (c + 1) * 512]), start=True, stop=True)
                    nc.vector.tensor_copy(out=cwr[:, c * 512 : (c + 1) * 512], in_=cps[:])
                dcw = dp.tile([N, N], f32, tag="dcw")
                nc.sync.dma_start(out=dcw[:].rearrange("a b -> (a b)")[None, :], in_=cwr[:])
                nc.sync.dma_start(out=dcw[:].rearrange("a b -> (a b)")[None, ::65], in_=ct["zrow"][:])
                CWM = wp.tile([N, N], f32, tag="CWM")
                nc.sync.dma_start(out=CWM[:], in_=dcw[:])

                # pos update: upd = CWM^T @ [P|1]
                upd = sm.tile([N, 4], f32, tag="sm")
                nc.tensor.matmul(out=upd[:], lhsT=CWM[:], rhs=Px[:], start=True, stop=True)
                upds = wp.tile([N, 4], f32, tag="upds")
                nc.vector.tensor_copy(out=upds[:], in_=upd[:])
                tmp = wp.tile([N, C], f32, tag="tmp")
                nc.vector.tensor_scalar_mul(out=tmp[:], in0=Px[:, 0:3], scalar1=upds[:, 3:4])
                nc.vector.tensor_tensor(out=Px[:, 0:3], in0=Px[:, 0:3], in1=upds[:, 0:3], op=add)
                nc.vector.tensor_tensor(out=Px[:, 0:3], in0=Px[:, 0:3], in1=tmp[:], op=sub)
                # refresh PT
                ptp = sm.tile([C, N], f32, tag="sm")
                nc.tensor.transpose(out=ptp[:], in_=Px[:, 0:3], identity=ct["ident"][:])
                nc.vector.tensor_copy(out=PT[:], in_=ptp[:])

                # msg_j = sum_i m[:, (i,j)]
                msg = wp.tile([H, N], f32r, tag="msg")
                nc.vector.tensor_reduce(out=msg[:], in_=m[:].rearrange("p (i j) -> p j i", i=N, j=N), axis=AX, op=add)
                nc.vector.tensor_tensor(out=msg[:], in0=msg[:], in1=m[:, ::65], op=sub)  # drop i==j phantom

                # node MLP
                nps = sm.tile([H, N], f32, tag="sm")
                nc.tensor.matmul(out=nps[:], lhsT=r(wsl("nw1a", l)), rhs=r(HT[:]), start=True, stop=False)
                nc.tensor.matmul(out=nps[:], lhsT=r(wsl("nw1b", l)), rhs=r(msg[:]), start=False, stop=True)
                u = wp.tile([H, N], f32r, tag="u")
                act_silu(u[:], nps[:], bsl(3, l), "z2")
                nps2 = sm.tile([H, N], f32, tag="sm")
                nc.tensor.matmul(out=nps2[:], lhsT=r(wsl("nw2", l)), rhs=r(u[:]), start=True, stop=True)
                nc.vector.tensor_tensor(out=HT[:], in0=HT[:], in1=nps2[:], op=add)
                nc.vector.tensor_scalar_add(out=HT[:], in0=HT[:], scalar1=bsl(4, l))

            # ---- finalize graph g: dx = P - P0, mean-center, scale ----
            p0 = wp.tile([N, C], f32, tag="p0")
            nc.sync.dma_start(out=p0[:], in_=dr["xin"][g])
            dxt = wp.tile([N, C], f32, tag="dxt")
            nc.vector.tensor_tensor(out=dxt[:], in0=Px[:, 0:3], in1=p0[:], op=sub)
            mean = sm.tile([1, C], f32, tag="sm")
            nc.tensor.matmul(out=mean[:], lhsT=ct["msc"][:, 0:1], rhs=dxt[:], start=True, stop=True)
            means = wp.tile([1, C], f32, tag="means")
            nc.vector.tensor_copy(out=means[:], in_=mean[:])
            mrep = sm.tile([N, C], f32, tag="sm")
            nc.tensor.matmul(out=mrep[:], lhsT=ct["inv64"][:], rhs=means[:], start=True, stop=True)
            nc.vector.tensor_tensor(out=dxt[:], in0=dxt[:], in1=mrep[:], op=sub)
            nc.vector.tensor_scalar_mul(out=dxt[:], in0=dxt[:], scalar1=ct["msc"][:, 1:2])
            nc.sync.dma_start(out=y[g], in_=dxt[:])

    nc.compile()
    return nc


def _get_nc(n_graphs, num_devices):
    key = (n_graphs, num_devices)
    if key not in _CACHE:
        _CACHE[key] = _build(n_graphs, num_devices)
    return _CACHE[key]


def make_in_maps(inputs, n_graphs=GPC, ncores=NCORES):
    consts = _prep_consts(inputs)
    x = np.asarray(inputs["x"], np.float32)
    in_maps = []
    for c in range(ncores):
        xs = x[c * n_graphs : (c + 1) * n_graphs].reshape(n_graphs, N, C)
        m = dict(consts)
        m["xin"] = np.ascontiguousarray(xs)
        m["xtin"] = np.ascontiguousarray(xs.transpose(0, 2, 1))
        in_maps.append(m)
    return in_maps


def kernel(**inputs) -> np.ndarray:
    from concourse.bass_utils import run_bass_kernel_spmd

    nc = _get_nc(GPC, NCORES)
    in_maps = make_in_maps(inputs)
    res = run_bass_kernel_spmd(nc, in_maps, core_ids=list(range(NCORES)), trace=False)
    outs = [res.results[c]["y"].reshape(GPC, N * C) for c in range(NCORES)]
    return np.concatenate(outs, axis=0).astype(np.float32)

